# revision 3
# baseline (speedup 1.0000x reference)
"""CrossNet kernel for Trainium2 (8 NeuronCores, pure data parallel).

Math (bias folded; bias==0 in practice):
    A_i   = 1 + x . w_i          (per-row, i = 0..2)
    T3    = (A_0*A_1 + beta1)*A_2 + beta2
    out   = x * T3 (+ b0+b1+b2)

Layout: per core x is [2048, 1024] viewed as [128, 16, 1024] — partition p
holds rows 16p..16p+15 (row r = p*16 + t).

Variants (env CROSSNET_VARIANT, default "ttr_bf16"):
  ttr_bf16 — SWDGE casting loads fp32->bf16; DVE fused tensor_tensor_reduce
             (bf16 ins, init 1.0, fp32 accum); ScalarE final mul (ACT scale,
             bf16 in -> fp32 out).
  stt_bf16 — same loads; DVE fused scalar_tensor_tensor bf16 (+1 folded into
             the T3 tail ops instead of reduce init).
  fp32     — plain HWDGE fp32 loads; DVE fused scalar_tensor_tensor fp32;
             ScalarE final mul fp32. (known-good op set, slower DVE)
  hybrid   — dots 0,1 fused on DVE (STT bf16, 1226ns/tile); dot 2 split:
             one chunked bf16 TT on DVE (2290ns/chunk, 2x mode) + per-tile
             ScalarE activation-accumulate reduce. Balances DVE ~53us /
             ScalarE ~45us under the ~46us DMA roofline.
  split    — all 3 dots as one chunked broadcast bf16 TT (x read 3x via
             stride-0 layer axis) + one segmented bf16->bf16 reduce per
             chunk on DVE. Fastest if the 1-src reduce hits a 2x/4x mode.
"""

import os

import numpy as np
import ml_dtypes

import concourse.bacc as bacc
import concourse.mybir as mybir
import concourse.tile as tile
from concourse.bass_utils import run_bass_kernel_spmd

BATCH, DIM, LAYERS = 16384, 1024, 3
NCORES = 8
ROWS = BATCH // NCORES   # 2048 rows per core
P = 128                  # SBUF partitions
SLOTS = ROWS // P        # 16 row-slots per partition
CHUNK = 4                # max row-slots per DMA/compute chunk
# uneven chunks: small first (faster pipeline start) and small last
# (shorter drain tail)
CHUNKS = [2, 4, 4, 3, 2, 1]
assert sum(CHUNKS) == SLOTS

F32 = mybir.dt.float32
BF16 = mybir.dt.bfloat16

mult = mybir.AluOpType.mult
add = mybir.AluOpType.add
Copy = mybir.ActivationFunctionType.Copy

VARIANT = "hybrid"


def _build(with_bias: bool, beta1: float, beta2: float):
    nc = bacc.Bacc("TRN2", target_bir_lowering=False, debug=False)
    bf16_path = VARIANT in ("ttr_bf16", "stt_bf16", "hybrid", "split")
    XDT = BF16 if bf16_path else F32

    x_d = nc.dram_tensor("x", [P, SLOTS * DIM], F32, kind="ExternalInput").ap()
    w_d = nc.dram_tensor("w", [P, LAYERS * DIM], XDT, kind="ExternalInput").ap()
    if with_bias:
        b3_d = nc.dram_tensor("b3", [P, DIM], XDT, kind="ExternalInput").ap()
    out_d = nc.dram_tensor("out", [P, SLOTS * DIM], F32, kind="ExternalOutput").ap()

    with tile.TileContext(nc) as tc, \
            tc.tile_pool(name="main", bufs=1) as pool, \
            tc.tile_pool(name="outs", bufs=4) as opool, \
            tc.tile_pool(name="p2", bufs=3) as p2pool:
        wb = pool.tile([P, LAYERS, DIM], XDT, name="wb", tag="wb")
        nc.sync.dma_start(wb[:], w_d.rearrange("p (l d) -> p l d", l=LAYERS))
        if with_bias:
            b3 = pool.tile([P, DIM], XDT, name="b3", tag="b3")
            nc.sync.dma_start(b3[:], b3_d[:])

        xb = pool.tile([P, SLOTS, DIM], XDT, name="xb", tag="xb")
        scr = pool.tile([P, DIM], XDT, name="scr", tag="scr")
        A = pool.tile([P, LAYERS, SLOTS], F32, name="A", tag="A")
        t2 = pool.tile([P, SLOTS], F32, name="t2", tag="t2")
        t3 = pool.tile([P, SLOTS], F32, name="t3", tag="t3")
        if VARIANT != "ttr_bf16":
            Ap = pool.tile([P, LAYERS, SLOTS], F32, name="Ap", tag="Ap")
        if VARIANT == "hybrid":
            sc_scr = pool.tile([P, DIM], XDT, name="sc_scr", tag="sc_scr")
        if VARIANT == "split":
            Ab = pool.tile([P, SLOTS, LAYERS], XDT, name="Ab", tag="Ab")
            Ap2 = pool.tile([P, SLOTS, LAYERS], F32, name="Ap2", tag="Ap2")

        xv = x_d.rearrange("p (s d) -> p s d", s=SLOTS)

        # loads up-front: the queue streams them back-to-back at HBM rate
        bounds = []
        s = 0
        for n in CHUNKS:
            bounds.append((s, s + n))
            s += n
        # finer-grained loads (2 slots = 1 MiB HBM read each): completion
        # semaphores fire sooner, so DVE never stalls waiting for a big
        # chunk to finish
        for l0 in range(0, SLOTS, 2):
            l1 = min(l0 + 2, SLOTS)
            if bf16_path:
                nc.gpsimd.dma_start(xb[:, l0:l1, :], xv[:, l0:l1, :])
            else:
                nc.sync.dma_start(xb[:, l0:l1, :], xv[:, l0:l1, :])

        for s0, s1 in bounds:
            if VARIANT == "split":
                cn = s1 - s0
                prod = p2pool.tile([P, cn, LAYERS, DIM], XDT,
                                   name="prod", tag="prod")
                xbc = xb[:, s0:s1, :].unsqueeze(2).broadcast_to(
                    [P, cn, LAYERS, DIM])
                wbc = wb.unsqueeze(1).broadcast_to([P, cn, LAYERS, DIM])
                nc.vector.tensor_mul(prod[:], xbc, wbc)
                with nc.allow_low_precision("bf16 dot, 2e-2 gate"):
                    nc.vector.reduce_sum(Ab[:, s0:s1, :], prod[:],
                                         axis=mybir.AxisListType.X)
                nc.vector.tensor_scalar_add(
                    Ap2[:, s0:s1, :], Ab[:, s0:s1, :], 1.0)
                nc.vector.tensor_mul(
                    t2[:, s0:s1], Ap2[:, s0:s1, 0], Ap2[:, s0:s1, 1])
                if beta1 != 0.0:
                    nc.vector.tensor_scalar_add(
                        t2[:, s0:s1], t2[:, s0:s1], beta1)
                nc.vector.tensor_mul(
                    t3[:, s0:s1], t2[:, s0:s1], Ap2[:, s0:s1, 2])
                if beta2 != 0.0:
                    nc.vector.tensor_scalar_add(
                        t3[:, s0:s1], t3[:, s0:s1], beta2)
                xo = opool.tile([P, cn, DIM], F32, name="xo", tag="xo")
                for j in range(cn):
                    t = s0 + j
                    if with_bias:
                        nc.vector.scalar_tensor_tensor(
                            xo[:, j, :], xb[:, t, :], t3[:, t:t + 1], b3[:],
                            op0=mult, op1=add)
                    else:
                        nc.scalar.activation(
                            xo[:, j, :], xb[:, t, :], Copy,
                            scale=t3[:, t:t + 1])
                nc.scalar.dma_start(
                    out_d[:, s0 * DIM:s1 * DIM],
                    xo.rearrange("p c d -> p (c d)"))
                continue
            last = s1 == SLOTS
            ndve = 2 if VARIANT == "hybrid" else LAYERS
            if VARIANT == "hybrid":
                # dot 2 first: the chunked bf16 multiply on DVE (2x mode)
                # unblocks ScalarE's accumulate-reduce chain while DVE is
                # still running this chunk's fused dots
                cn = s1 - s0
                prod2 = p2pool.tile([P, cn, DIM], XDT, name="prod2", tag="prod2")
                wb2 = wb[:, 2, :].unsqueeze(1).broadcast_to([P, cn, DIM])
                nc.vector.tensor_mul(prod2[:], xb[:, s0:s1, :], wb2)
                for j in range(cn):
                    t = s0 + j
                    nc.scalar.activation(
                        sc_scr[:], prod2[:, j, :], Copy,
                        accum_out=A[:, 2, t:t + 1],
                    )
            for t in range(s0, s1):
                for i in range(ndve):
                    if VARIANT == "ttr_bf16":
                        nc.vector.tensor_tensor_reduce(
                            scr[:], xb[:, t, :], wb[:, i, :], 1.0, 1.0,
                            op0=mult, op1=add,
                            accum_out=A[:, i, t:t + 1],
                        )
                    else:
                        nc.vector.scalar_tensor_tensor(
                            scr[:], xb[:, t, :], 1.0, wb[:, i, :],
                            op0=mult, op1=mult,
                            accum_out=A[:, i, t:t + 1],
                        )
            # T3 = (A0*A1 + beta1)*A2 + beta2 over the chunk
            for r0, r1 in [(s0, s1)]:
                if VARIANT == "ttr_bf16":
                    Av = A  # accumulators already seeded with 1.0
                else:
                    Av = Ap
                    nc.vector.tensor_scalar_add(
                        Av[:, :, r0:r1], A[:, :, r0:r1], 1.0)
                nc.vector.tensor_mul(t2[:, r0:r1], Av[:, 0, r0:r1], Av[:, 1, r0:r1])
                if beta1 != 0.0:
                    nc.vector.tensor_scalar_add(t2[:, r0:r1], t2[:, r0:r1], beta1)
                nc.vector.tensor_mul(t3[:, r0:r1], t2[:, r0:r1], Av[:, 2, r0:r1])
                if beta2 != 0.0:
                    nc.vector.tensor_scalar_add(t3[:, r0:r1], t3[:, r0:r1], beta2)

            cn = s1 - s0
            xo = opool.tile([P, cn, DIM], F32, name="xo", tag="xo")
            for j in range(cn):
                t = s0 + j
                if with_bias:
                    nc.vector.scalar_tensor_tensor(
                        xo[:, j, :], xb[:, t, :], t3[:, t:t + 1], b3[:],
                        op0=mult, op1=add,
                    )
                else:
                    nc.scalar.activation(
                        xo[:, j, :], xb[:, t, :], Copy, scale=t3[:, t:t + 1]
                    )
                if last:
                    # per-slot stores on the drain tail: the final store is
                    # 0.5 MiB instead of the whole chunk
                    nc.scalar.dma_start(
                        out_d[:, t * DIM:(t + 1) * DIM], xo[:, j, :])
            if not last:
                nc.scalar.dma_start(
                    out_d[:, s0 * DIM:s1 * DIM],
                    xo.rearrange("p c d -> p (c d)"))

    nc.compile()
    return nc


def prepare(x: np.ndarray, kernels: np.ndarray, bias: np.ndarray):
    """Build the Bass program and the per-core input maps."""
    x = np.ascontiguousarray(x, dtype=np.float32)
    kernels = np.asarray(kernels, dtype=np.float32)
    bias = np.asarray(bias, dtype=np.float32)

    beta1 = float(bias[0] @ kernels[1])
    beta2 = float((bias[0] + bias[1]) @ kernels[2])
    b3 = bias.sum(axis=0)
    with_bias = bool(np.any(b3 != 0.0))

    nc = _build(with_bias, beta1, beta2)

    wdt = (ml_dtypes.bfloat16
           if VARIANT in ("ttr_bf16", "stt_bf16", "hybrid") else np.float32)
    w_rep = np.ascontiguousarray(np.broadcast_to(
        kernels.reshape(1, LAYERS * DIM), (P, LAYERS * DIM)).astype(wdt))
    in_maps = []
    for c in range(NCORES):
        m = {
            "x": x[c * ROWS:(c + 1) * ROWS].reshape(P, SLOTS * DIM),
            "w": w_rep,
        }
        if with_bias:
            m["b3"] = np.ascontiguousarray(
                np.broadcast_to(b3, (P, DIM)).astype(wdt))
        in_maps.append(m)
    return nc, in_maps


def kernel(x: np.ndarray, kernels: np.ndarray, bias: np.ndarray) -> np.ndarray:
    nc, in_maps = prepare(x, kernels, bias)
    res = run_bass_kernel_spmd(nc, in_maps, list(range(NCORES)))
    return np.concatenate(
        [r["out"].reshape(ROWS, DIM) for r in res.results], axis=0)
